# revision 8
# baseline (speedup 1.0000x reference)
"""Trainium2 Bass kernel for nn_CausalSelfAttention_40810779247124.

Head-sharded (tensor-parallel) causal self-attention prefill across 8
NeuronCores: 2 heads per core. All SBUF operands are bf16 (psum f32),
which keeps every matmul at 1 cycle/row (f32r pays 4x below 256-wide
moving dims), halves DMA bytes, and lands ~5e-3 relative error.

Per core:
  phase 1: QKV projection for its 2 heads. Q/K kept resident in SBUF as
           [e, tok] bf16; V produced PE-transposed as [tok, e] bf16.
  phase 2: attention with scores transposed: scT[t,s] = K^T Q. Causal
           work is exact at 128-row granularity: for the 4 diagonal
           t-chunks of each 512-query tile the matmul/exp/z/wv are
           narrowed to the s-columns that need them (17408 free-cycles
           per (batch,head) -- the causal minimum). exp on ACT,
           denominator z via ones-column matmul on PE (the fastest
           partition-reducer), 1/z broadcast across partitions on the
           idle GPSIMD engine.
  phase 3: out-projection partial for this core's d-slice, interleaved
           per 512-token step one step behind attention (hides the
           finalize chain), psum->sbuf copies alternate DVE/ACT, bf16
           partials DMA'd out; the all-reduce over cores is the host
           summing 8 partials.

The host verifies mask/cache_pos match causal prefill and falls back to
a numpy reference otherwise.
"""

import sys

sys.path.insert(0, "/opt/trn_rl_repo")

import numpy as np

B = 2
S = 2048
T = 4096
NS = 2048          # n_state
H = 16
DH = 128
NCORES = 8
HPC = H // NCORES  # heads per core = 2
DPC = HPC * DH     # d-slice per core = 256
TOK = B * S        # 4096 tokens across batches
SCALE = 1.0 / float(np.sqrt(DH))

_CACHED = {}


def _build_program():
    import concourse.bacc as bacc
    import concourse.bass as bass
    import concourse.tile as tile
    from concourse import mybir

    bf16 = mybir.dt.bfloat16
    f32 = mybir.dt.float32
    f32r = mybir.dt.float32r
    EXP = mybir.ActivationFunctionType.Exp
    COPY = mybir.ActivationFunctionType.Copy

    nc = bacc.Bacc()

    xT = nc.dram_tensor("xT", [NS, TOK], bf16, kind="ExternalInput")
    wT = nc.dram_tensor("wT", [NS, 6 * DH], bf16, kind="ExternalInput")
    woutT = nc.dram_tensor("woutT", [DPC, NS], bf16, kind="ExternalInput")
    cmask = nc.dram_tensor("cmask", [DH, DH], bf16, kind="ExternalInput")
    outp = nc.dram_tensor("outp", [TOK, NS], bf16, kind="ExternalOutput")

    NT = TOK // 512   # 8 tok-tiles of 512
    NK = NS // 128    # 16 contraction chunks

    with tile.TileContext(nc) as tc:
        with (
            tc.tile_pool(name="constp", bufs=1) as constp,
            tc.tile_pool(name="vresp", bufs=1) as vresp,
            tc.tile_pool(name="qkresp", bufs=1) as qkresp,
            tc.tile_pool(name="woutp", bufs=1) as woutp,
        ):
            # tri[t, s] = 1.0 if s >= t (within-diag causal mask).
            # Column 127 is all ones -> doubles as the z ones-column.
            tri = constp.tile([DH, DH], bf16)
            ones_col = tri[:, 127:128]

            # V resident: v_res[p, c, e] = V[c*128+p, e] (tok-major)
            v_res = vresp.tile([128, TOK // 128, DPC], bf16)
            # Q,K resident [e-block(q0,q1,k0,k1), tok]
            qk_res = qkresp.tile([128, 4, TOK], bf16)

            # ---------------- phase 1: QKV projection ----------------
            with (
                tc.tile_pool(name="wp", bufs=1) as wp,
                tc.tile_pool(name="xp", bufs=3) as xp,
                tc.tile_pool(name="qkv_ps", bufs=4, space="PSUM") as qkv_ps,
                tc.tile_pool(name="v_ps", bufs=4, space="PSUM") as v_ps,
            ):
                nc.scalar.dma_start(out=tri[:, :], in_=cmask[:, :])
                w_sb = wp.tile([128, NK, 6 * DH], bf16)
                wout_sb = woutp.tile([128, HPC, NS], bf16)
                for h in range(HPC):
                    nc.scalar.dma_start(
                        out=wout_sb[:, h, :],
                        in_=woutT[128 * h : 128 * (h + 1), :],
                    )
                # Warm the ACT exp table while the PE is busy on QKV.
                warm = constp.tile([1, 1], f32)
                nc.scalar.activation(
                    out=warm, in_=tri[0:1, 0:1], func=EXP, scale=1.0
                )

                for a in range(NT):
                    pss = [
                        qkv_ps.tile([128, 512], f32, tag="qkv", name=f"qkv{m}")
                        for m in range(4)
                    ]
                    vps = [
                        v_ps.tile([128, 256], f32, tag="vps", name=f"vps{t}")
                        for t in range(4)
                    ]
                    for half in range(2):
                        x_sb = xp.tile([128, NK // 2, 512], bf16, tag="x_sb")
                        for kc in range(NK // 2):
                            nc.sync.dma_start(
                                out=x_sb[:, kc, :],
                                in_=xT[
                                    1024 * half + 128 * kc : 1024 * half
                                    + 128 * (kc + 1),
                                    512 * a : 512 * (a + 1),
                                ],
                            )
                        for kc in range(NK // 2):
                            kk = half * (NK // 2) + kc
                            if a == 0:
                                if kk == 0:
                                    for mm in range(6):
                                        nc.scalar.dma_start(
                                            out=w_sb[
                                                :, kk, 128 * mm : 128 * (mm + 1)
                                            ],
                                            in_=wT[
                                                128 * kk : 128 * (kk + 1),
                                                128 * mm : 128 * (mm + 1),
                                            ],
                                        )
                                else:
                                    nc.scalar.dma_start(
                                        out=w_sb[:, kk, :],
                                        in_=wT[128 * kk : 128 * (kk + 1), :],
                                    )
                            for m in range(4):
                                nc.tensor.matmul(
                                    pss[m],
                                    w_sb[:, kk, 128 * m : 128 * (m + 1)],
                                    x_sb[:, kc, :],
                                    start=(kk == 0),
                                    stop=(kk == NK - 1),
                                )
                            for t in range(4):
                                nc.tensor.matmul(
                                    vps[t],
                                    x_sb[:, kc, 128 * t : 128 * (t + 1)],
                                    w_sb[:, kk, 512:768],
                                    start=(kk == 0),
                                    stop=(kk == NK - 1),
                                )
                    for m in range(4):
                        nc.vector.tensor_copy(
                            out=qk_res[:, m, 512 * a : 512 * (a + 1)],
                            in_=pss[m],
                        )
                    for t in range(4):
                        nc.vector.tensor_copy(
                            out=v_res[:, 4 * a + t, :], in_=vps[t]
                        )

            # ------- phases 2+3: attention + out-projection, interleaved -------
            with (
                tc.tile_pool(name="ptp", bufs=6) as ptp,
                tc.tile_pool(name="zrp", bufs=2) as zrp,
                tc.tile_pool(name="zbp", bufs=2) as zbp,
                tc.tile_pool(name="wvnp", bufs=4) as wvnp,
                tc.tile_pool(name="ostage", bufs=3) as ostage,
                tc.tile_pool(name="sc_ps", bufs=3, space="PSUM") as sc_ps,
                tc.tile_pool(name="wv_ps", bufs=2, space="PSUM") as wv_ps,
                tc.tile_pool(name="z_ps", bufs=1, space="PSUM") as z_ps,
                tc.tile_pool(name="o_ps", bufs=2, space="PSUM") as o_ps,
            ):
                def attn_tile(b, h, ast, wvn):
                    q_sb = qk_res[:, h, S * b + 512 * ast : S * b + 512 * (ast + 1)]
                    wv = wv_ps.tile([128, 512], f32, tag="wv")
                    z = z_ps.tile([1, 512], f32, tag="z")
                    nfull = 4 * ast
                    nj = nfull + 4
                    for j in range(nj):
                        p = j - nfull
                        lo = 0 if p < 0 else 128 * p  # causal narrowing
                        sc = sc_ps.tile([128, 512], f32, tag="sc")
                        nc.tensor.matmul(
                            sc[:, lo:],
                            qk_res[:, 2 + h, S * b + 128 * j : S * b + 128 * (j + 1)],
                            q_sb[:, lo:],
                            start=True,
                            stop=True,
                        )
                        pt = ptp.tile([128, 512], bf16, tag="pt")
                        nc.scalar.activation(
                            out=pt[:, lo:], in_=sc[:, lo:], func=EXP, scale=SCALE
                        )
                        if p >= 0:
                            nc.vector.tensor_mul(
                                pt[:, lo : lo + 128], pt[:, lo : lo + 128], tri
                            )
                        nc.tensor.matmul(
                            z[:, lo:],
                            ones_col,
                            pt[:, lo:],
                            start=(j == 0),
                            stop=(j == nj - 1),
                            skip_group_check=True,
                        )
                        nc.tensor.matmul(
                            wv[:, lo:],
                            v_res[:, 16 * b + j, 128 * h : 128 * (h + 1)],
                            pt[:, lo:],
                            start=(j == 0),
                            stop=(j == nj - 1),
                            skip_group_check=True,
                        )
                    zr = zrp.tile([1, 512], f32r, tag="zr")
                    with nc.allow_low_precision(
                        reason="f32r is bit-identical to f32"
                    ):
                        nc.vector.reciprocal(out=zr, in_=z)
                    zbs = zbp.tile([128, 512], f32r, tag="zbs")
                    nc.gpsimd.partition_broadcast(zbs, zr, channels=128)
                    nc.vector.tensor_mul(
                        wvn[:, 512 * ast : 512 * (ast + 1)], wv, zbs
                    )

                def outproj(b, ast, wvn_pair):
                    for tk in range(4):
                        toff = 512 * ast + 128 * tk
                        ost = ostage.tile([128, NS], bf16, tag="ost")
                        for n in range(4):
                            ops = o_ps.tile([128, 512], f32, tag="ops")
                            for h in range(HPC):
                                nc.tensor.matmul(
                                    ops,
                                    wvn_pair[h][:, toff : toff + 128],
                                    wout_sb[:, h, 512 * n : 512 * (n + 1)],
                                    start=(h == 0),
                                    stop=(h == HPC - 1),
                                )
                            dst = ost[:, 512 * n : 512 * (n + 1)]
                            if n % 2 == 0:
                                nc.vector.tensor_copy(out=dst, in_=ops)
                            else:
                                nc.gpsimd.tensor_copy(out=dst, in_=ops)
                        nc.sync.dma_start(
                            out=outp[S * b + toff : S * b + toff + 128, :],
                            in_=ost,
                        )

                jobs = []
                for b in range(B):
                    wvn_pair = [
                        wvnp.tile([128, S], bf16, tag="wvn", name=f"wvn{b}_{h}")
                        for h in range(HPC)
                    ]
                    for ast in range(4):
                        attn_tile(b, 0, ast, wvn_pair[0])
                        if jobs:
                            outproj(*jobs.pop(0))
                        attn_tile(b, 1, ast, wvn_pair[1])
                        jobs.append((b, ast, wvn_pair))
                while jobs:
                    outproj(*jobs.pop(0))

    nc.compile()
    return nc


def _causal_fastpath_ok(mask, cache_pos):
    if cache_pos.shape != (S,) or not np.array_equal(
        np.asarray(cache_pos), np.arange(S, dtype=np.int64).astype(cache_pos.dtype)
    ):
        return False
    m = np.asarray(mask).reshape(S, T)
    rows = np.arange(S)[:, None]
    cols = np.arange(T)[None, :]
    return np.array_equal(m, cols <= rows)


def _numpy_fallback(input_ids, mask, cache_pos, w_qkv, w_out, k_cache, v_cache):
    x = np.asarray(input_ids, dtype=np.float32)
    qkv = np.einsum("bsd,ed->bse", x, np.asarray(w_qkv, np.float32))
    q, k, v = np.split(qkv, 3, axis=-1)

    def heads(t):
        return t.reshape(B, S, H, DH).transpose(0, 2, 1, 3)

    q, k, v = heads(q), heads(k), heads(v)
    kf = np.array(k_cache, np.float32)
    vf = np.array(v_cache, np.float32)
    kf[:, :, np.asarray(cache_pos)] = k
    vf[:, :, np.asarray(cache_pos)] = v
    sc = np.einsum("bhsd,bhtd->bhst", q, kf) * SCALE
    sc = np.where(np.asarray(mask), sc, np.finfo(np.float32).min)
    sc = sc - sc.max(axis=-1, keepdims=True)
    p = np.exp(sc)
    p = p / p.sum(axis=-1, keepdims=True)
    wv = np.einsum("bhst,bhtd->bhsd", p, vf)
    wv = wv.transpose(0, 2, 1, 3).reshape(B, S, NS)
    return np.einsum("bsd,ed->bse", wv, np.asarray(w_out, np.float32))


def _build_cmask_host():
    # tri[t, s] = 1.0 if s >= t; column 127 is all-ones (z ones-column).
    t = np.arange(DH)[:, None]
    s = np.arange(DH)[None, :]
    return (s >= t).astype(np.float32)


def _run_on_device(in_maps, trace=False):
    from concourse.bass_utils import run_bass_kernel_spmd

    if "nc" not in _CACHED:
        _CACHED["nc"] = _build_program()
    nc = _CACHED["nc"]
    return run_bass_kernel_spmd(
        nc, in_maps, core_ids=list(range(NCORES)), trace=trace
    )


def _prep_in_maps(input_ids, w_qkv, w_out):
    import ml_dtypes

    bf = ml_dtypes.bfloat16
    x2d = np.ascontiguousarray(
        np.asarray(input_ids, np.float32).reshape(TOK, NS).T
    ).astype(bf)  # [NS, TOK]
    cm = _build_cmask_host().astype(bf)
    wq = np.asarray(w_qkv, np.float32)
    wo = np.asarray(w_out, np.float32)
    in_maps = []
    for c in range(NCORES):
        lo, hi = c * DPC, (c + 1) * DPC
        w_slice = np.concatenate(
            [wq[lo:hi], wq[NS + lo : NS + hi], wq[2 * NS + lo : 2 * NS + hi]],
            axis=0,
        )  # [768, NS] (q,k,v rows for this core's heads)
        wT_c = np.ascontiguousarray(w_slice.T).astype(bf)        # [NS, 768]
        woutT_c = np.ascontiguousarray(wo[:, lo:hi].T).astype(bf)  # [DPC, NS]
        in_maps.append({"xT": x2d, "wT": wT_c, "woutT": woutT_c, "cmask": cm})
    return in_maps


def kernel(input_ids, mask, cache_pos, w_qkv, w_out, k_cache, v_cache):
    if not _causal_fastpath_ok(mask, cache_pos):
        return _numpy_fallback(
            input_ids, mask, cache_pos, w_qkv, w_out, k_cache, v_cache
        )
    in_maps = _prep_in_maps(input_ids, w_qkv, w_out)
    res = _run_on_device(in_maps)
    out = np.zeros((TOK, NS), np.float32)
    for r in res.results:
        out += np.asarray(r["outp"], dtype=np.float32)
    return out.reshape(B, S, NS)
